# revision 1
# baseline (speedup 1.0000x reference)
"""Trainium2 Bass kernel for nn_Encoder_82274393522442.

PointNet-style encoder: 5 pointwise conv (1x1) layers 3->64->128->256->256->1024
with ReLU between, then global max-pool over N=8192 points. B=32, out [32,1024].

Strategy:
- Data-parallel over batch: 8 cores x 4 batches each. No collectives; host concat.
- On-chip layout: channels on partitions, tokens (points) on the free dim.
  Token tile = 512 (one PSUM bank of fp32).
- Matmuls in float32r (fp32 storage, tf32-like multiply): 1 cycle/row on the PE
  (same speed as bf16, ~16x better precision), fp32 PSUM accumulation.
- ReLU+bias fused on ScalarE (ACT) reading PSUM, writing f32r SBUF tiles.
- Max-pool folded in as free-dim tensor_reduce(max) on VectorE straight from
  L5's PSUM, into per-(batch,tile) columns; final small reduce + bias at the end.
"""

import numpy as np

import concourse.bass as bass
import concourse.mybir as mybir
import concourse.tile as tile
from concourse import bacc
from concourse.bass import ts
from concourse.bass_utils import run_bass_kernel_spmd

F32 = mybir.dt.float32
F32R = mybir.dt.float32r
RELU = mybir.ActivationFunctionType.Relu
MAX = mybir.AluOpType.max
AX_X = mybir.AxisListType.X

B, C0, N, Z = 32, 3, 8192, 1024
NCORES = 8
PB = B // NCORES  # batches per core = 4
T = 512  # token tile (one fp32 PSUM bank)
NT = N // T  # 16 token tiles per batch


def build_bass():
    nc = bacc.Bacc("TRN2", target_bir_lowering=False, debug=False, num_devices=NCORES)

    x = nc.dram_tensor("x", [PB, C0, N], F32R, kind="ExternalInput")
    w1t = nc.dram_tensor("w1t", [C0, 64], F32R, kind="ExternalInput")
    w2t = nc.dram_tensor("w2t", [64, 128], F32R, kind="ExternalInput")
    w3t = nc.dram_tensor("w3t", [128, 256], F32R, kind="ExternalInput")
    w4t = nc.dram_tensor("w4t", [128, 2, 256], F32R, kind="ExternalInput")
    w5t = nc.dram_tensor("w5t", [128, 2, 1024], F32R, kind="ExternalInput")
    bias = nc.dram_tensor("bias", [128, 6], F32, kind="ExternalInput")
    b5t = nc.dram_tensor("b5t", [128, 8], F32, kind="ExternalInput")
    out = nc.dram_tensor("out", [PB, Z], F32, kind="ExternalOutput")

    with tile.TileContext(nc) as tc:
        with (
            tc.tile_pool(name="wp", bufs=1) as wp,
            tc.tile_pool(name="xp", bufs=2) as xp,
            tc.tile_pool(name="ap", bufs=3) as ap_,
            tc.tile_pool(name="mp", bufs=2) as mp,
            tc.tile_pool(name="op", bufs=2) as op_,
            tc.tile_pool(name="p1p", bufs=1, space="PSUM") as p1p,
            tc.tile_pool(name="p2p", bufs=1, space="PSUM") as p2p,
            tc.tile_pool(name="p34p", bufs=1, space="PSUM") as p34p,
            tc.tile_pool(name="p5p", bufs=2, space="PSUM") as p5p,
        ):
            tw1 = wp.tile([C0, 64], F32R)
            tw2 = wp.tile([64, 128], F32R)
            tw3 = wp.tile([128, 256], F32R)
            tw4 = wp.tile([128, 2, 256], F32R)
            tw5 = wp.tile([128, 2, 1024], F32R)
            tbias = wp.tile([128, 6], F32)
            tb5 = wp.tile([128, 8], F32)
            nc.sync.dma_start(tw1, w1t.ap())
            nc.sync.dma_start(tw2, w2t.ap())
            nc.sync.dma_start(tw3, w3t.ap())
            nc.sync.dma_start(tw4, w4t.ap())
            nc.sync.dma_start(tw5, w5t.ap())
            nc.sync.dma_start(tbias, bias.ap())
            nc.sync.dma_start(tb5, b5t.ap())

            for b in range(PB):
                xb = xp.tile([C0, N], F32R, tag="xb", name="xb")
                nc.sync.dma_start(xb, x.ap()[b])
                # per-(tile, zgroup) maxima for this batch
                mxb = mp.tile([128, NT, 8], F32, tag="mx", name="mxb")

                for t in range(NT):
                    xt = xb[:, ts(t, T)]
                    # L1: 3 -> 64
                    p1 = p1p.tile([64, T], F32, tag="p1", name="p1")
                    nc.tensor.matmul(p1, tw1, xt, start=True, stop=True)
                    a1 = ap_.tile([64, T], F32R, tag="a1", name="a1")
                    nc.scalar.activation(a1, p1, RELU, bias=tbias[:64, 0:1])
                    # L2: 64 -> 128
                    p2 = p2p.tile([128, T], F32, tag="p2", name="p2")
                    nc.tensor.matmul(p2, tw2, a1, start=True, stop=True)
                    a2 = ap_.tile([128, T], F32R, tag="a2", name="a2")
                    nc.scalar.activation(a2, p2, RELU, bias=tbias[:, 1:2])
                    # L3: 128 -> 256
                    p3 = p34p.tile([128, 2, T], F32, tag="p34", name="p3")
                    for g in range(2):
                        nc.tensor.matmul(
                            p3[:, g, :], tw3[:, ts(g, 128)], a2, start=True, stop=True
                        )
                    a3 = ap_.tile([128, 2, T], F32R, tag="a3", name="a3")
                    for g in range(2):
                        nc.scalar.activation(
                            a3[:, g, :], p3[:, g, :], RELU, bias=tbias[:, 2 + g : 3 + g]
                        )
                    # L4: 256 -> 256 (accumulate over 2 K-halves)
                    p4 = p34p.tile([128, 2, T], F32, tag="p34", name="p4")
                    for o in range(2):
                        for g in range(2):
                            nc.tensor.matmul(
                                p4[:, o, :],
                                tw4[:, g, ts(o, 128)],
                                a3[:, g, :],
                                start=(g == 0),
                                stop=(g == 1),
                            )
                    a4 = ap_.tile([128, 2, T], F32R, tag="a4", name="a4")
                    for o in range(2):
                        nc.scalar.activation(
                            a4[:, o, :], p4[:, o, :], RELU, bias=tbias[:, 4 + o : 5 + o]
                        )
                    # L5: 256 -> 1024, in 4 chunks of 2 z-groups; fold max over
                    # tokens straight out of PSUM
                    for c in range(4):
                        p5 = p5p.tile([128, 2, T], F32, tag="p5", name="p5")
                        for zi in range(2):
                            z = 2 * c + zi
                            for g in range(2):
                                nc.tensor.matmul(
                                    p5[:, zi, :],
                                    tw5[:, g, ts(z, 128)],
                                    a4[:, g, :],
                                    start=(g == 0),
                                    stop=(g == 1),
                                )
                        nc.vector.tensor_reduce(
                            mxb[:, t, 2 * c : 2 * c + 2], p5, axis=AX_X, op=MAX
                        )

                # batch epilogue: max over the 16 tile-maxima, add b5, store
                mxr = op_.tile([128, 8], F32, tag="mxr", name="mxr")
                nc.vector.tensor_reduce(
                    mxr, mxb.rearrange("p t z -> p z t"), axis=AX_X, op=MAX
                )
                ob = op_.tile([128, 8], F32, tag="ob", name="ob")
                nc.vector.tensor_add(ob, mxr, tb5)
                nc.sync.dma_start(out.ap()[b].rearrange("(z p) -> p z", p=128), ob)

    nc.finalize()
    return nc


_NC_CACHE = None


def _get_nc():
    global _NC_CACHE
    if _NC_CACHE is None:
        _NC_CACHE = build_bass()
    return _NC_CACHE


def _prep_in_maps(inputs):
    f32 = np.float32
    x = np.ascontiguousarray(np.asarray(inputs["x"], dtype=f32))  # [32, 3, 8192]
    W = [np.asarray(inputs[f"W{i}"], dtype=f32) for i in range(1, 6)]
    bvec = [np.asarray(inputs[f"b{i}"], dtype=f32) for i in range(1, 6)]

    w1t = np.ascontiguousarray(W[0].T)  # [3, 64]
    w2t = np.ascontiguousarray(W[1].T)  # [64, 128]
    w3t = np.ascontiguousarray(W[2].T)  # [128, 256]
    # W4.T is [256(in), 256(out)]; -> [in128, g, out] with g the K-half
    w4t = np.ascontiguousarray(W[3].T.reshape(2, 128, 256).transpose(1, 0, 2))
    w5t = np.ascontiguousarray(W[4].T.reshape(2, 128, 1024).transpose(1, 0, 2))

    bias = np.zeros((128, 6), dtype=f32)
    bias[:64, 0] = bvec[0]
    bias[:, 1] = bvec[1]
    bias[:, 2] = bvec[2][:128]
    bias[:, 3] = bvec[2][128:]
    bias[:, 4] = bvec[3][:128]
    bias[:, 5] = bvec[3][128:]
    b5t = np.ascontiguousarray(bvec[4].reshape(8, 128).T)

    shared = {
        "w1t": w1t,
        "w2t": w2t,
        "w3t": w3t,
        "w4t": w4t,
        "w5t": w5t,
        "bias": bias,
        "b5t": b5t,
    }
    in_maps = []
    for c in range(NCORES):
        m = dict(shared)
        m["x"] = x[c * PB : (c + 1) * PB]
        in_maps.append(m)
    return in_maps


def run(inputs, **spmd_kwargs):
    """Run on all 8 cores; returns (output [32,1024] f32, BassKernelResults)."""
    nc = _get_nc()
    in_maps = _prep_in_maps(inputs)
    res = run_bass_kernel_spmd(nc, in_maps, core_ids=list(range(NCORES)), **spmd_kwargs)
    out = np.concatenate([res.results[c]["out"] for c in range(NCORES)], axis=0)
    return out.astype(np.float32), res


def kernel(**inputs):
    out, _ = run(inputs)
    return out


# revision 2
# speedup vs baseline: 1.0195x; 1.0195x over previous
"""Trainium2 Bass kernel for nn_Encoder_82274393522442.

PointNet-style encoder: 5 pointwise conv (1x1) layers 3->64->128->256->256->1024
with ReLU between, then global max-pool over N=8192 points. B=32, out [32,1024].

Strategy:
- Data-parallel over batch: 8 cores x 4 batches each. No collectives; host concat.
- On-chip layout: channels on partitions, tokens (points) on the free dim.
  Token tile = 512 (one PSUM bank of fp32).
- Matmuls in float32r (fp32 storage, tf32-like multiply): 1 cycle/row on the PE
  (same speed as bf16, ~16x better precision), fp32 PSUM accumulation.
- ReLU+bias fused on ScalarE (ACT) reading PSUM, writing f32r SBUF tiles.
- Max-pool folded in as free-dim tensor_reduce(max) on VectorE straight from
  L5's PSUM, into per-(batch,tile) columns; final small reduce + bias at the end.
"""

import numpy as np

import concourse.bass as bass
import concourse.mybir as mybir
import concourse.tile as tile
from concourse import bacc
from concourse.bass import ts
from concourse.bass_utils import run_bass_kernel_spmd

F32 = mybir.dt.float32
F32R = mybir.dt.float32r
RELU = mybir.ActivationFunctionType.Relu
MAX = mybir.AluOpType.max
AX_X = mybir.AxisListType.X

B, C0, N, Z = 32, 3, 8192, 1024
NCORES = 8
PB = B // NCORES  # batches per core = 4
T = 512  # token tile (one fp32 PSUM bank)
NT = N // T  # 16 token tiles per batch


def build_bass():
    nc = bacc.Bacc("TRN2", target_bir_lowering=False, debug=False, num_devices=NCORES)

    x = nc.dram_tensor("x", [PB, C0, N], F32R, kind="ExternalInput")
    w1t = nc.dram_tensor("w1t", [C0, 64], F32R, kind="ExternalInput")
    w2t = nc.dram_tensor("w2t", [64, 128], F32R, kind="ExternalInput")
    w3t = nc.dram_tensor("w3t", [128, 256], F32R, kind="ExternalInput")
    w4t = nc.dram_tensor("w4t", [128, 2, 256], F32R, kind="ExternalInput")
    w5t = nc.dram_tensor("w5t", [128, 2, 1024], F32R, kind="ExternalInput")
    bias = nc.dram_tensor("bias", [128, 6], F32, kind="ExternalInput")
    b5t = nc.dram_tensor("b5t", [128, 8], F32, kind="ExternalInput")
    out = nc.dram_tensor("out", [PB, Z], F32, kind="ExternalOutput")

    with tile.TileContext(nc) as tc:
        with (
            tc.tile_pool(name="wp", bufs=1) as wp,
            tc.tile_pool(name="xp", bufs=2) as xp,
            tc.tile_pool(name="ap", bufs=3) as ap_,
            tc.tile_pool(name="mp", bufs=2) as mp,
            tc.tile_pool(name="op", bufs=2) as op_,
            tc.tile_pool(name="p12p", bufs=2, space="PSUM") as p12p,
            tc.tile_pool(name="p34p", bufs=1, space="PSUM") as p34p,
            tc.tile_pool(name="p5p", bufs=2, space="PSUM") as p5p,
        ):
            tw1 = wp.tile([C0, 64], F32R)
            tw2 = wp.tile([64, 128], F32R)
            tw3 = wp.tile([128, 256], F32R)
            tw4 = wp.tile([128, 2, 256], F32R)
            tw5 = wp.tile([128, 2, 1024], F32R)
            tbias = wp.tile([128, 6], F32)
            tb5 = wp.tile([128, 8], F32)
            # critical-path-first DMA emission: L1-L3 weights + first x chunks,
            # then the big tail weights
            nc.sync.dma_start(tw1, w1t.ap())
            nc.sync.dma_start(tbias, bias.ap())
            nc.sync.dma_start(tw2, w2t.ap())
            nc.sync.dma_start(tw3, w3t.ap())

            NXC = N // 4  # x DMA chunk = 4 token tiles

            def load_x(b):
                xb = xp.tile([C0, N], F32R, tag="xb", name="xb")
                for j in range(4):
                    nc.sync.dma_start(
                        xb[:, ts(j, NXC)], x.ap()[b][:, ts(j, NXC)]
                    )
                return xb

            xb = load_x(0)
            nc.sync.dma_start(tw4, w4t.ap())
            nc.sync.dma_start(tb5, b5t.ap())
            nc.sync.dma_start(tw5, w5t.ap())

            # software pipeline: iteration i runs L1-L4 of tile i interleaved
            # with the four L5+max chunks of tile i-1, so the PE never waits
            # on ACT relu latency or DVE reduce backpressure.
            state = None  # (a4, mxb, t, b) of previous tile

            def emit_chunk(st, c):
                if st is None:
                    return
                a4p, mxbp, tp, bp = st
                p5 = p5p.tile([128, 2, T], F32, tag="p5", name="p5")
                for zi in range(2):
                    z = 2 * c + zi
                    for g in range(2):
                        nc.tensor.matmul(
                            p5[:, zi, :],
                            tw5[:, g, ts(z, 128)],
                            a4p[:, g, :],
                            start=(g == 0),
                            stop=(g == 1),
                        )
                nc.vector.tensor_reduce(
                    mxbp[:, tp, 2 * c : 2 * c + 2], p5, axis=AX_X, op=MAX
                )
                if c == 3 and tp == NT - 1:
                    # batch epilogue: max over 16 tile-maxima, add b5, store
                    mxr = op_.tile([128, 8], F32, tag="mxr", name="mxr")
                    nc.vector.tensor_reduce(
                        mxr, mxbp.rearrange("p t z -> p z t"), axis=AX_X, op=MAX
                    )
                    ob = op_.tile([128, 8], F32, tag="ob", name="ob")
                    nc.vector.tensor_add(ob, mxr, tb5)
                    nc.sync.dma_start(
                        out.ap()[bp].rearrange("(z p) -> p z", p=128), ob
                    )

            mxb = None
            for i in range(PB * NT + 1):
                prev = state
                if i < PB * NT:
                    b, t = divmod(i, NT)
                    if t == 0:
                        if b > 0:
                            xb = load_x(b)
                        mxb = mp.tile([128, NT, 8], F32, tag="mx", name="mxb")
                    xt = xb[:, ts(t, T)]
                    # L1: 3 -> 64
                    p1 = p12p.tile([64, T], F32, tag="p12", name="p1")
                    nc.tensor.matmul(p1, tw1, xt, start=True, stop=True)
                    a1 = ap_.tile([64, T], F32R, tag="a1", name="a1")
                    nc.scalar.activation(a1, p1, RELU, bias=tbias[:64, 0:1])
                    emit_chunk(prev, 0)
                    # L2: 64 -> 128
                    p2 = p12p.tile([128, T], F32, tag="p12", name="p2")
                    nc.tensor.matmul(p2, tw2, a1, start=True, stop=True)
                    a2 = ap_.tile([128, T], F32R, tag="a2", name="a2")
                    nc.scalar.activation(a2, p2, RELU, bias=tbias[:, 1:2])
                    emit_chunk(prev, 1)
                    # L3: 128 -> 256
                    p3 = p34p.tile([128, 2, T], F32, tag="p34", name="p3")
                    for g in range(2):
                        nc.tensor.matmul(
                            p3[:, g, :], tw3[:, ts(g, 128)], a2, start=True, stop=True
                        )
                    a3 = ap_.tile([128, 2, T], F32R, tag="a3", name="a3")
                    for g in range(2):
                        nc.scalar.activation(
                            a3[:, g, :], p3[:, g, :], RELU, bias=tbias[:, 2 + g : 3 + g]
                        )
                    emit_chunk(prev, 2)
                    # L4: 256 -> 256 (accumulate over 2 K-halves); relu each
                    # output half right after its own matmuls
                    p4 = p34p.tile([128, 2, T], F32, tag="p34", name="p4")
                    a4 = ap_.tile([128, 2, T], F32R, tag="a4", name="a4")
                    for o in range(2):
                        for g in range(2):
                            nc.tensor.matmul(
                                p4[:, o, :],
                                tw4[:, g, ts(o, 128)],
                                a3[:, g, :],
                                start=(g == 0),
                                stop=(g == 1),
                            )
                        nc.scalar.activation(
                            a4[:, o, :], p4[:, o, :], RELU, bias=tbias[:, 4 + o : 5 + o]
                        )
                    emit_chunk(prev, 3)
                    state = (a4, mxb, t, b)
                else:
                    for c in range(4):
                        emit_chunk(prev, c)
                    state = None

    nc.finalize()
    return nc


_NC_CACHE = None


def _get_nc():
    global _NC_CACHE
    if _NC_CACHE is None:
        _NC_CACHE = build_bass()
    return _NC_CACHE


def _prep_in_maps(inputs):
    f32 = np.float32
    x = np.ascontiguousarray(np.asarray(inputs["x"], dtype=f32))  # [32, 3, 8192]
    W = [np.asarray(inputs[f"W{i}"], dtype=f32) for i in range(1, 6)]
    bvec = [np.asarray(inputs[f"b{i}"], dtype=f32) for i in range(1, 6)]

    w1t = np.ascontiguousarray(W[0].T)  # [3, 64]
    w2t = np.ascontiguousarray(W[1].T)  # [64, 128]
    w3t = np.ascontiguousarray(W[2].T)  # [128, 256]
    # W4.T is [256(in), 256(out)]; -> [in128, g, out] with g the K-half
    w4t = np.ascontiguousarray(W[3].T.reshape(2, 128, 256).transpose(1, 0, 2))
    w5t = np.ascontiguousarray(W[4].T.reshape(2, 128, 1024).transpose(1, 0, 2))

    bias = np.zeros((128, 6), dtype=f32)
    bias[:64, 0] = bvec[0]
    bias[:, 1] = bvec[1]
    bias[:, 2] = bvec[2][:128]
    bias[:, 3] = bvec[2][128:]
    bias[:, 4] = bvec[3][:128]
    bias[:, 5] = bvec[3][128:]
    b5t = np.ascontiguousarray(bvec[4].reshape(8, 128).T)

    shared = {
        "w1t": w1t,
        "w2t": w2t,
        "w3t": w3t,
        "w4t": w4t,
        "w5t": w5t,
        "bias": bias,
        "b5t": b5t,
    }
    in_maps = []
    for c in range(NCORES):
        m = dict(shared)
        m["x"] = x[c * PB : (c + 1) * PB]
        in_maps.append(m)
    return in_maps


def run(inputs, **spmd_kwargs):
    """Run on all 8 cores; returns (output [32,1024] f32, BassKernelResults)."""
    nc = _get_nc()
    in_maps = _prep_in_maps(inputs)
    res = run_bass_kernel_spmd(nc, in_maps, core_ids=list(range(NCORES)), **spmd_kwargs)
    out = np.concatenate([res.results[c]["out"] for c in range(NCORES)], axis=0)
    return out.astype(np.float32), res


def kernel(**inputs):
    out, _ = run(inputs)
    return out


# revision 6
# speedup vs baseline: 1.1441x; 1.1222x over previous
"""Trainium2 Bass kernel for nn_Encoder_82274393522442.

PointNet-style encoder: 5 pointwise conv (1x1) layers 3->64->128->256->256->1024
with ReLU between, then global max-pool over N=8192 points. B=32, out [32,1024].

Strategy:
- Data-parallel over batch: 8 cores x 4 batches each. No collectives; host concat.
- On-chip layout: channels on partitions, tokens (points) on the free dim.
  Token tile = 512 (one PSUM bank of fp32).
- Matmuls in float32r (fp32 storage, tf32-like multiply): 1 cycle/row on the PE
  (same speed as bf16, ~16x better precision), fp32 PSUM accumulation.
- ReLU+bias fused on ScalarE (ACT) reading PSUM, writing f32r SBUF tiles.
- Max-pool folded in as free-dim tensor_reduce(max) on VectorE straight from
  L5's PSUM, into per-(batch,tile) columns; final small reduce + bias at the end.
"""

import numpy as np

import concourse.bass as bass
import concourse.mybir as mybir
import concourse.tile as tile
from concourse import bacc
from concourse.bass import ts
from concourse.bass_utils import run_bass_kernel_spmd

F32 = mybir.dt.float32
F32R = mybir.dt.float32r
RELU = mybir.ActivationFunctionType.Relu
MAX = mybir.AluOpType.max
AX_X = mybir.AxisListType.X

B, C0, N, Z = 32, 3, 8192, 1024
NCORES = 8
PB = B // NCORES  # batches per core = 4
T = 512  # token tile (one fp32 PSUM bank)
NT = N // T  # 16 token tiles per batch


def build_bass():
    nc = bacc.Bacc("TRN2", target_bir_lowering=False, debug=False, num_devices=NCORES)

    x = nc.dram_tensor("x", [PB, C0, N], F32R, kind="ExternalInput")
    w1t = nc.dram_tensor("w1t", [C0, 64], F32R, kind="ExternalInput")
    w2t = nc.dram_tensor("w2t", [64, 128], F32R, kind="ExternalInput")
    w3t = nc.dram_tensor("w3t", [128, 256], F32R, kind="ExternalInput")
    w4t = nc.dram_tensor("w4t", [128, 2, 256], F32R, kind="ExternalInput")
    w5t = nc.dram_tensor("w5t", [128, 2, 1024], F32R, kind="ExternalInput")
    bias = nc.dram_tensor("bias", [128, 6], F32, kind="ExternalInput")
    b5t = nc.dram_tensor("b5t", [128, 8], F32, kind="ExternalInput")
    out = nc.dram_tensor("out", [PB, Z], F32, kind="ExternalOutput")

    with tile.TileContext(nc) as tc:
        with (
            tc.tile_pool(name="wp", bufs=1) as wp,
            tc.tile_pool(name="xp", bufs=2) as xp,
            tc.tile_pool(name="ap", bufs=3) as ap_,
            tc.tile_pool(name="mp", bufs=2) as mp,
            tc.tile_pool(name="op", bufs=2) as op_,
            tc.tile_pool(name="p12p", bufs=2, space="PSUM") as p12p,
            tc.tile_pool(name="p34p", bufs=1, space="PSUM") as p34p,
            tc.tile_pool(name="p5p", bufs=2, space="PSUM") as p5p,
        ):
            tw1 = wp.tile([C0, 64], F32R)
            tw2 = wp.tile([64, 128], F32R)
            tw3 = wp.tile([128, 256], F32R)
            tw4 = wp.tile([128, 2, 256], F32R)
            tw5 = wp.tile([128, 2, 1024], F32R)
            tbias = wp.tile([128, 6], F32)
            tb5 = wp.tile([128, 8], F32)
            # critical-path-first DMA emission: L1-L3 weights + first x chunks,
            # then the big tail weights
            nc.sync.dma_start(tw1, w1t.ap())
            nc.sync.dma_start(tbias, bias.ap())
            nc.sync.dma_start(tw2, w2t.ap())
            nc.sync.dma_start(tw3, w3t.ap())

            NXC = N // 4  # x DMA chunk = 4 token tiles

            def load_x(b):
                xb = xp.tile([C0, N], F32R, tag="xb", name="xb")
                for j in range(4):
                    nc.sync.dma_start(
                        xb[:, ts(j, NXC)], x.ap()[b][:, ts(j, NXC)]
                    )
                return xb

            xb = load_x(0)
            nc.sync.dma_start(tw4, w4t.ap())
            nc.sync.dma_start(tb5, b5t.ap())
            nc.sync.dma_start(tw5, w5t.ap())

            # software pipeline: iteration i runs L1-L4 of tile i interleaved
            # with the four L5+max chunks of tile i-2, so the PE never waits
            # on ACT relu latency or DVE reduce backpressure.
            state = None  # (a4, mxb, t, b) of tile i-1
            state2 = None  # (a4, mxb, t, b) of tile i-2

            def emit_chunk(st, c):
                if st is None:
                    return
                a4p, mxbp, tp, bp = st
                p5 = p5p.tile([128, 2, T], F32, tag="p5", name="p5")
                for zi in range(2):
                    z = 2 * c + zi
                    for g in range(2):
                        nc.tensor.matmul(
                            p5[:, zi, :],
                            tw5[:, g, ts(z, 128)],
                            a4p[:, g, :],
                            start=(g == 0),
                            stop=(g == 1),
                        )
                nc.vector.tensor_reduce(
                    mxbp[:, tp, 2 * c : 2 * c + 2], p5, axis=AX_X, op=MAX
                )
                if c == 3 and tp == NT - 1:
                    # batch epilogue: max over 16 tile-maxima, add b5, store
                    mxr = op_.tile([128, 8], F32, tag="mxr", name="mxr")
                    nc.vector.tensor_reduce(
                        mxr, mxbp.rearrange("p t z -> p z t"), axis=AX_X, op=MAX
                    )
                    ob = op_.tile([128, 8], F32, tag="ob", name="ob")
                    nc.vector.tensor_add(ob, mxr, tb5)
                    nc.sync.dma_start(
                        out.ap()[bp].rearrange("(z p) -> p z", p=128), ob
                    )

            mxb = None
            for i in range(PB * NT + 2):
                prev = state2
                if i < PB * NT:
                    b, t = divmod(i, NT)
                    if t == 0:
                        if b > 0:
                            xb = load_x(b)
                        mxb = mp.tile([128, NT, 8], F32, tag="mx", name="mxb")
                    xt = xb[:, ts(t, T)]
                    # L1: 3 -> 64
                    p1 = p12p.tile([64, T], F32, tag="p12", name="p1")
                    nc.tensor.matmul(p1, tw1, xt, start=True, stop=True)
                    a1 = ap_.tile([64, T], F32R, tag="a1", name="a1")
                    nc.scalar.activation(a1, p1, RELU, bias=tbias[:64, 0:1])
                    emit_chunk(prev, 0)
                    # L2: 64 -> 128
                    p2 = p12p.tile([128, T], F32, tag="p12", name="p2")
                    nc.tensor.matmul(p2, tw2, a1, start=True, stop=True)
                    a2 = ap_.tile([128, T], F32R, tag="a2", name="a2")
                    nc.scalar.activation(a2, p2, RELU, bias=tbias[:, 1:2])
                    emit_chunk(prev, 1)
                    # L3: 128 -> 256
                    p3 = p34p.tile([128, 2, T], F32, tag="p34", name="p3")
                    for g in range(2):
                        nc.tensor.matmul(
                            p3[:, g, :], tw3[:, ts(g, 128)], a2, start=True, stop=True
                        )
                    a3 = ap_.tile([128, 2, T], F32R, tag="a3", name="a3")
                    for g in range(2):
                        nc.scalar.activation(
                            a3[:, g, :], p3[:, g, :], RELU, bias=tbias[:, 2 + g : 3 + g]
                        )
                    emit_chunk(prev, 2)
                    # L4: 256 -> 256 (accumulate over 2 K-halves); relu each
                    # output half right after its own matmuls
                    p4 = p34p.tile([128, 2, T], F32, tag="p34", name="p4")
                    a4 = ap_.tile([128, 2, T], F32R, tag="a4", name="a4", bufs=4)
                    for o in range(2):
                        for g in range(2):
                            nc.tensor.matmul(
                                p4[:, o, :],
                                tw4[:, g, ts(o, 128)],
                                a3[:, g, :],
                                start=(g == 0),
                                stop=(g == 1),
                            )
                        nc.scalar.activation(
                            a4[:, o, :], p4[:, o, :], RELU, bias=tbias[:, 4 + o : 5 + o]
                        )
                    emit_chunk(prev, 3)
                    state2 = state
                    state = (a4, mxb, t, b)
                else:
                    for c in range(4):
                        emit_chunk(prev, c)
                    state2 = state
                    state = None

    nc.finalize()
    return nc


_NC_CACHE = None


def _get_nc():
    global _NC_CACHE
    if _NC_CACHE is None:
        _NC_CACHE = build_bass()
    return _NC_CACHE


def _prep_in_maps(inputs):
    f32 = np.float32
    x = np.ascontiguousarray(np.asarray(inputs["x"], dtype=f32))  # [32, 3, 8192]
    W = [np.asarray(inputs[f"W{i}"], dtype=f32) for i in range(1, 6)]
    bvec = [np.asarray(inputs[f"b{i}"], dtype=f32) for i in range(1, 6)]

    w1t = np.ascontiguousarray(W[0].T)  # [3, 64]
    w2t = np.ascontiguousarray(W[1].T)  # [64, 128]
    w3t = np.ascontiguousarray(W[2].T)  # [128, 256]
    # W4.T is [256(in), 256(out)]; -> [in128, g, out] with g the K-half
    w4t = np.ascontiguousarray(W[3].T.reshape(2, 128, 256).transpose(1, 0, 2))
    w5t = np.ascontiguousarray(W[4].T.reshape(2, 128, 1024).transpose(1, 0, 2))

    bias = np.zeros((128, 6), dtype=f32)
    bias[:64, 0] = bvec[0]
    bias[:, 1] = bvec[1]
    bias[:, 2] = bvec[2][:128]
    bias[:, 3] = bvec[2][128:]
    bias[:, 4] = bvec[3][:128]
    bias[:, 5] = bvec[3][128:]
    b5t = np.ascontiguousarray(bvec[4].reshape(8, 128).T)

    shared = {
        "w1t": w1t,
        "w2t": w2t,
        "w3t": w3t,
        "w4t": w4t,
        "w5t": w5t,
        "bias": bias,
        "b5t": b5t,
    }
    in_maps = []
    for c in range(NCORES):
        m = dict(shared)
        m["x"] = x[c * PB : (c + 1) * PB]
        in_maps.append(m)
    return in_maps


def run(inputs, **spmd_kwargs):
    """Run on all 8 cores; returns (output [32,1024] f32, BassKernelResults)."""
    nc = _get_nc()
    in_maps = _prep_in_maps(inputs)
    res = run_bass_kernel_spmd(nc, in_maps, core_ids=list(range(NCORES)), **spmd_kwargs)
    out = np.concatenate([res.results[c]["out"] for c in range(NCORES)], axis=0)
    return out.astype(np.float32), res


def kernel(**inputs):
    out, _ = run(inputs)
    return out
